# revision 14
# baseline (speedup 1.0000x reference)
"""Trainium2 Bass kernel for nn_MinimalRNNCell.

Reference math (fp32):
    z_t = W_in x_t + b_in
    u_t = sigmoid(Wg_h h_{t-1} + Wg_z z_t + b_g)
    h_t = u_t * h_{t-1} + (1-u_t) * z_t
    y_t = W_out h_t + b_out
    output = y[:, batch=-1, :]  -> [T, O]   (only batch element 63 matters!)

Strategy (fixed-point iteration + hardware prefix scan, s-substitution):
  * Only sample 63 of the batch affects the output -> compute just that one.
  * With m := h - z and s_j := m_j + Delta_{j+1} (Delta_j := z_{j-1} - z_j)
    the update becomes   s_j = u_j s_{j-1} + Delta_{j+1},
    a first-order linear recurrence with ITERATION-STATIC coefficients --
    exactly the DVE's tensor_tensor_scan (state = u*state + d) with data1
    fixed to the precomputed Delta.  The gate argument also collapses:
        Wg_h m_{j-1} + Wg_z z_j + Wg_h z_{j-1} + b_g
      = Wg_h s_{j-1} + (Wg_z + Wg_h) z_j + b_g = Wg_h s_{j-1} + Q_j
    so each fixed-point round is just TWO matmuls (folded Q from x, plus
    Wg_h s), one sigmoid, and one scan per column chunk.  Round 0 uses
    s ~ Delta (i.e. m ~ 0) as the initial estimate via the same code path.
    The u->h coupling is weak (|dsigma|<=1/4, Wg_h entries ~0.06): each
    round contracts the error ~10x; three rounds reach ~8e-4 rel err in
    all-fp16, far under the 2e-2 gate.  m is recovered once at the end
    (m_j = u_j s_{j-1}) for the output matmul.
  * 8 cores each own 512 contiguous timesteps; each chunk restarts from
    m=0 with a W=16-column warmup prefix (error ~0.5^16, negligible).
    No serial per-timestep loop anywhere.
  * Host-folded fp16 weights: q65 = [((Wg_z+Wg_h) W_in)^T ; (Wg_z+Wg_h)
    b_in + b_g], winp/winn = +-[W_in^T ; b_in] (Delta straight from x),
    wox65 = [(W_out W_in)^T ; W_out b_in + b_out] (folds z and all biases
    into the output matmul: y = x~^T wox65 + m^T W_out^T, no z tensor).
    The ones-row of x~ carries the biases (zeroed where global t < 0).
  * s is ping-pong buffered per round so the two 264-column chunks
    decouple; PSUM tiles ring through banks; sigmoid ACT table preloads
    under the input DMAs.  The serial critical path per round is just
    matmul -> sigmoid -> scan.
"""

import numpy as np

import concourse.bass as bass
import concourse.mybir as mybir
import concourse.tile as tile
from concourse import bacc
from concourse.bass_utils import run_bass_kernel_spmd

# problem constants (hardcoded per harness contract)
T, I, H, O = 4096, 64, 128, 64
NCORES = 8
TLOC = T // NCORES          # timesteps per core
W = 16                      # warmup columns per core chunk
NZ = 2 + W + TLOC           # x columns (1 leading for z_{j-1}, 1 trailing)
N = W + TLOC                # scan columns j = 1..N
CH = N // 2                 # column chunk (264)
NITER = 3                   # fixed-point rounds (round 0 seeds s ~ Delta)

_C16_COLS = {
    "q65": (0, 128), "winp": (128, 128), "winn": (256, 128),
    "wghh": (384, 128), "woutT": (512, 64), "wox65": (576, 64),
}
NC16 = 640

FP32 = mybir.dt.float32
FP16 = mybir.dt.float16
AF = mybir.ActivationFunctionType
OP = mybir.AluOpType


def _build_program():
    nc = bacc.Bacc()

    xT = nc.dram_tensor("xT", [I + 1, NZ], FP16, kind="ExternalInput")
    c16 = nc.dram_tensor("c16", [128, NC16], FP16, kind="ExternalInput")
    y = nc.dram_tensor("y", [TLOC, O], FP32, kind="ExternalOutput")

    with tile.TileContext(nc) as tc:
        with (
            tc.tile_pool(name="singles", bufs=1) as singles,
            tc.tile_pool(name="state", bufs=2) as state,
            tc.tile_pool(name="psum", bufs=6, space="PSUM") as psum,
            tc.tile_pool(name="psum_y", bufs=1, space="PSUM") as psum_y,
        ):
            # ---- sigmoid ACT table preload (~1.3us) under the input DMAs ----
            junk = singles.tile([128, 1], FP32)
            nc.vector.memset(junk, 0.0)
            junk_sig = singles.tile([128, 1], FP32)
            nc.scalar.activation(junk_sig, junk, AF.Sigmoid)

            # ---- input DMAs (SP + Pool queues; ACT stays on the table) ----
            x_sb = singles.tile([I + 1, NZ], FP16)
            c_sb = singles.tile([128, NC16], FP16)
            nc.sync.dma_start(out=x_sb, in_=xT[:, :])
            nc.gpsimd.dma_start(out=c_sb, in_=c16[:, :])

            def cs(nm, rows=128):
                c0, n_ = _C16_COLS[nm]
                return c_sb[0:rows, c0:c0 + n_]

            q65 = cs("q65", I + 1)
            winp = cs("winp", I + 1)
            winn = cs("winn", I + 1)
            wghh = cs("wghh")
            woutT = cs("woutT")
            wox65 = cs("wox65", I + 1)

            # ---- Delta_j = z_{j-1} - z_j for j=1..N+1 -> d16 col j-1
            # (PE -> PSUM, DVE downcast to fp16 SBUF) ----
            d16 = singles.tile([H, N + 1], FP16)
            for c0, cn in ((0, CH + 1), (CH + 1, CH)):
                ps_d = psum.tile([H, CH + 1], FP32, tag="ps")
                nc.tensor.matmul(ps_d[:, 0:cn], winp, x_sb[:, c0:c0 + cn],
                                 start=True, stop=False)
                nc.tensor.matmul(ps_d[:, 0:cn], winn,
                                 x_sb[:, 1 + c0:1 + c0 + cn],
                                 start=False, stop=True)
                nc.vector.tensor_copy(d16[:, c0:c0 + cn], ps_d[:, 0:cn])

            # ---- s ping-pong buffers; col j = s_j, col 0 = Delta_1 (m_0=0)
            sbuf = []
            for mi in range(2):
                sb = singles.tile([H, N + 1], FP16, name=f"s16_{mi}")
                nc.vector.tensor_copy(sb[:, 0:1], d16[:, 0:1])
                sbuf.append(sb)

            # ---- fixed-point rounds ----
            ufin = [None, None]
            for it in range(NITER):
                scur = sbuf[it % 2]
                sprev = d16 if it == 0 else sbuf[(it + 1) % 2]
                for ci in range(2):
                    c0 = ci * CH
                    ps_u = psum.tile([H, CH], FP32, tag="ps")
                    nc.tensor.matmul(ps_u, q65, x_sb[:, 1 + c0:1 + c0 + CH],
                                     start=True, stop=False)
                    nc.tensor.matmul(ps_u, wghh, sprev[:, c0:c0 + CH],
                                     start=False, stop=True)
                    u16 = state.tile([H, CH], FP16, tag=f"u{ci}")
                    nc.scalar.activation(u16, ps_u, AF.Sigmoid)
                    ufin[ci] = u16
                    init = d16[:, 0:1] if ci == 0 else scur[:, c0:c0 + 1]
                    nc.vector.tensor_tensor_scan(
                        scur[:, 1 + c0:1 + c0 + CH], u16,
                        d16[:, 1 + c0:1 + c0 + CH], init, OP.mult, OP.add)

            # ---- m_j = u_j s_{j-1}; y = x~^T wox65 + m^T W_out^T ----
            sfin = sbuf[(NITER - 1) % 2]
            mfin = singles.tile([H, N + 1], FP16)
            ysb = singles.tile([128, TLOC // 128, O], FP32)
            ps_y = psum_y.tile([128, TLOC // 128, O], FP32, tag="ps_y")
            y_view = y.rearrange("(b p) o -> p b o", p=128)
            # block 0 (cols 17..145) only needs chunk 0's m; blocks 1-3 need
            # both chunks -> split 1 + 3 so the first DMA launches early
            for half, blocks in ((0, (0,)), (1, (1, 2, 3))):
                c0 = half * CH
                nc.vector.tensor_mul(mfin[:, 1 + c0:1 + c0 + CH],
                                     ufin[half], sfin[:, c0:c0 + CH])
                for b in blocks:
                    xs = slice(W + 1 + b * 128, W + 1 + (b + 1) * 128)
                    nc.tensor.matmul(ps_y[:, b, :], x_sb[:, xs], wox65,
                                     start=True, stop=False)
                    nc.tensor.matmul(ps_y[:, b, :], mfin[:, xs], woutT,
                                     start=False, stop=True)
                hb = slice(blocks[0], blocks[-1] + 1)
                nc.scalar.activation(ysb[:, hb, :], ps_y[:, hb, :], AF.Copy)
                nc.sync.dma_start(out=y_view[:, hb, :], in_=ysb[:, hb, :])

    nc.compile()
    return nc


_PROGRAM = None


def _get_program():
    global _PROGRAM
    if _PROGRAM is None:
        _PROGRAM = _build_program()
    return _PROGRAM


def _prepare_in_maps(inputs):
    x = np.ascontiguousarray(np.asarray(inputs["inputs"], dtype=np.float64)[63])
    W_in = np.asarray(inputs["W_in"], dtype=np.float64)
    b_in = np.asarray(inputs["b_in"], dtype=np.float64)
    W_g = np.asarray(inputs["W_g"], dtype=np.float64)
    b_g = np.asarray(inputs["b_g"], dtype=np.float64)
    W_out = np.asarray(inputs["W_out"], dtype=np.float64)
    b_out = np.asarray(inputs["b_out"], dtype=np.float64)

    Wg_h = W_g[:, :H]
    Wg_z = W_g[:, H:]

    c16 = np.zeros((128, NC16), np.float16)

    def put(name, v):
        c0, n_ = _C16_COLS[name]
        c16[:v.shape[0], c0:c0 + n_] = v.astype(np.float16)

    Wq = Wg_z + Wg_h
    put("q65", np.concatenate([(Wq @ W_in).T, (Wq @ b_in + b_g)[None, :]], 0))
    win65 = np.concatenate([W_in.T, b_in[None, :]], 0)
    put("winp", win65)
    put("winn", -win65)
    put("wghh", Wg_h.T)
    put("woutT", W_out.T)
    put("wox65", np.concatenate([(W_out @ W_in).T,
                                 (W_out @ b_in + b_out)[None, :]], 0))

    # x padded with W+1 leading and 1 trailing zero rows plus a ones-row
    # that carries the biases through the matmuls (zeroed where t < 0 or
    # t >= T)
    xpad = np.zeros((W + 1 + T + 1, I + 1), np.float16)
    xpad[W + 1:W + 1 + T, :I] = x.astype(np.float16)
    xpad[W + 1:W + 1 + T, I] = 1.0

    in_maps = []
    for k in range(NCORES):
        lo = k * TLOC
        xk = np.ascontiguousarray(xpad[lo:lo + NZ].T)
        in_maps.append({"xT": xk, "c16": c16})
    return in_maps


def _run(in_maps, **kwargs):
    nc = _get_program()
    return run_bass_kernel_spmd(nc, in_maps, list(range(NCORES)), **kwargs)


def kernel(**inputs):
    res = _run(_prepare_in_maps(inputs))
    y = np.concatenate([res.results[k]["y"] for k in range(NCORES)], axis=0)
    return np.ascontiguousarray(y.astype(np.float32))


if __name__ == "__main__":
    d = np.load("/root/problem/inputs.npz")
    out = kernel(**{k: d[k] for k in d.files})
    exp = np.load("/root/problem/expected.npy")
    err = np.abs(out - exp).max()
    print("absmax err vs expected:", err, " rel:", err / np.abs(exp).max())


# revision 20
# speedup vs baseline: 1.0132x; 1.0132x over previous
"""Trainium2 Bass kernel for nn_MinimalRNNCell.

Reference math (fp32):
    z_t = W_in x_t + b_in
    u_t = sigmoid(Wg_h h_{t-1} + Wg_z z_t + b_g)
    h_t = u_t * h_{t-1} + (1-u_t) * z_t
    y_t = W_out h_t + b_out
    output = y[:, batch=-1, :]  -> [T, O]   (only batch element 63 matters!)

Strategy (fixed-point iteration + hardware prefix scan, s-substitution):
  * Only sample 63 of the batch affects the output -> compute just that one.
  * With m := h - z and s_j := m_j + Delta_{j+1} (Delta_j := z_{j-1} - z_j)
    the update becomes   s_j = u_j s_{j-1} + Delta_{j+1},
    a first-order linear recurrence with ITERATION-STATIC coefficients --
    exactly the DVE's tensor_tensor_scan (state = u*state + d) with data1
    fixed to the precomputed Delta.  The gate argument also collapses:
        Wg_h m_{j-1} + Wg_z z_j + Wg_h z_{j-1} + b_g
      = Wg_h s_{j-1} + (Wg_z + Wg_h) z_j + b_g = Wg_h s_{j-1} + Q_j
    so each fixed-point round is just TWO matmuls (folded Q from x, plus
    Wg_h s), one sigmoid, and one scan per column chunk.  Round 0 uses
    s ~ Delta (i.e. m ~ 0) as the initial estimate via the same code path.
    The u->h coupling is weak (|dsigma|<=1/4, Wg_h entries ~0.06): each
    round contracts the error ~10x; three rounds reach ~8e-4 rel err in
    all-fp16, far under the 2e-2 gate.  m is recovered once at the end
    (m_j = u_j s_{j-1}) for the output matmul.
  * 8 cores each own 512 contiguous timesteps; each chunk restarts from
    m=0 with a W=16-column warmup prefix (error ~0.5^16, negligible).
    No serial per-timestep loop anywhere.
  * Host-folded fp16 weights: q65 = [((Wg_z+Wg_h) W_in)^T ; (Wg_z+Wg_h)
    b_in + b_g], winp/winn = +-[W_in^T ; b_in] (Delta straight from x),
    wox65 = [(W_out W_in)^T ; W_out b_in + b_out] (folds z and all biases
    into the output matmul: y = x~^T wox65 + m^T W_out^T, no z tensor).
    The ones-row of x~ carries the biases (zeroed where global t < 0).
  * s is ping-pong buffered per round so the two 264-column chunks
    decouple; PSUM tiles ring through banks; sigmoid ACT table preloads
    under the input DMAs.  The serial critical path per round is just
    matmul -> sigmoid -> scan.
"""

import numpy as np

import concourse.bass as bass
import concourse.mybir as mybir
import concourse.tile as tile
from concourse import bacc
from concourse.bass_utils import run_bass_kernel_spmd

# problem constants (hardcoded per harness contract)
T, I, H, O = 4096, 64, 128, 64
NCORES = 8
TLOC = T // NCORES          # timesteps per core
W = 16                      # warmup columns per core chunk
NZ = 2 + W + TLOC           # x columns (1 leading for z_{j-1}, 1 trailing)
N = W + TLOC                # scan columns j = 1..N
CH = N // 2                 # column chunk (264)
NITER = 2                   # fixed-point rounds (round 0 seeds s ~ Delta)
NCRIT = 384                 # c16 cols in the first (critical) weight DMA

_C16_COLS = {
    "q65": (0, 128), "winp": (128, 128), "winn": (256, 128),
    "wghh": (384, 128), "woutT": (512, 64), "wox65": (576, 64),
}
NC16 = 640

FP32 = mybir.dt.float32
FP16 = mybir.dt.float16
AF = mybir.ActivationFunctionType
OP = mybir.AluOpType


def _build_program():
    nc = bacc.Bacc()

    xT = nc.dram_tensor("xT", [I + 1, NZ], FP16, kind="ExternalInput")
    c16 = nc.dram_tensor("c16", [128, NC16], FP16, kind="ExternalInput")
    # y laid out [partition, block*O] so each partition's DMA is one
    # contiguous 1KB descriptor; the host unshuffles to [TLOC, O]
    y = nc.dram_tensor("y", [128, (TLOC // 128) * O], FP32,
                       kind="ExternalOutput")

    with tile.TileContext(nc) as tc:
        with (
            tc.tile_pool(name="singles", bufs=1) as singles,
            tc.tile_pool(name="state", bufs=2) as state,
            tc.tile_pool(name="psum", bufs=6, space="PSUM") as psum,
            tc.tile_pool(name="psum_y", bufs=1, space="PSUM") as psum_y,
        ):
            # ---- sigmoid ACT table preload (~1.3us) under the input DMAs ----
            junk = singles.tile([128, 1], FP32)
            nc.vector.memset(junk, 0.0)
            junk_sig = singles.tile([128, 1], FP32)
            nc.scalar.activation(junk_sig, junk, AF.Sigmoid)

            # ---- input DMAs (SP + Pool queues; ACT stays on the table) ----
            x_sb = singles.tile([I + 1, NZ], FP16)
            c_sb = singles.tile([128, NC16], FP16)
            nc.sync.dma_start(out=x_sb, in_=xT[:, :])
            # weights split in two DMAs: the first carries everything the
            # prologue needs, so one straggler packet can't stall the start
            nc.gpsimd.dma_start(out=c_sb[:, 0:NCRIT], in_=c16[:, 0:NCRIT])
            nc.gpsimd.dma_start(out=c_sb[:, NCRIT:NC16], in_=c16[:, NCRIT:NC16])

            def cs(nm, rows=128):
                c0, n_ = _C16_COLS[nm]
                return c_sb[0:rows, c0:c0 + n_]

            q65 = cs("q65", I + 1)
            winp = cs("winp", I + 1)
            winn = cs("winn", I + 1)
            wghh = cs("wghh")
            woutT = cs("woutT")
            wox65 = cs("wox65", I + 1)

            # ---- Delta_j = z_{j-1} - z_j for j=1..N+1 -> d16 col j-1
            # (PE -> PSUM, DVE downcast to fp16 SBUF) ----
            d16 = singles.tile([H, N + 1], FP16)
            for c0, cn in ((0, CH + 1), (CH + 1, CH)):
                ps_d = psum.tile([H, CH + 1], FP32, tag="ps")
                nc.tensor.matmul(ps_d[:, 0:cn], winp, x_sb[:, c0:c0 + cn],
                                 start=True, stop=False)
                nc.tensor.matmul(ps_d[:, 0:cn], winn,
                                 x_sb[:, 1 + c0:1 + c0 + cn],
                                 start=False, stop=True)
                nc.vector.tensor_copy(d16[:, c0:c0 + cn], ps_d[:, 0:cn])

            # ---- s ping-pong buffers; col j = s_j, col 0 = Delta_1 (m_0=0)
            sbuf = []
            for mi in range(2):
                sb = singles.tile([H, N + 1], FP16, name=f"s16_{mi}")
                nc.vector.tensor_copy(sb[:, 0:1], d16[:, 0:1])
                sbuf.append(sb)

            # ---- fixed-point rounds ----
            ufin = [None, None]
            for it in range(NITER):
                scur = sbuf[it % 2]
                sprev = d16 if it == 0 else sbuf[(it + 1) % 2]
                for ci in range(2):
                    c0 = ci * CH
                    ps_u = psum.tile([H, CH], FP32, tag="ps")
                    nc.tensor.matmul(ps_u, q65, x_sb[:, 1 + c0:1 + c0 + CH],
                                     start=True, stop=False)
                    nc.tensor.matmul(ps_u, wghh, sprev[:, c0:c0 + CH],
                                     start=False, stop=True)
                    u16 = state.tile([H, CH], FP16, tag=f"u{ci}")
                    nc.scalar.activation(u16, ps_u, AF.Sigmoid)
                    ufin[ci] = u16
                    init = d16[:, 0:1] if ci == 0 else scur[:, c0:c0 + 1]
                    nc.vector.tensor_tensor_scan(
                        scur[:, 1 + c0:1 + c0 + CH], u16,
                        d16[:, 1 + c0:1 + c0 + CH], init, OP.mult, OP.add)

            # ---- m_j = u_j s_{j-1}; y = x~^T wox65 + m^T W_out^T ----
            sfin = sbuf[(NITER - 1) % 2]
            mfin = singles.tile([H, N + 1], FP16)
            ysb = singles.tile([128, TLOC // 128, O], FP32)
            ps_y = psum_y.tile([128, TLOC // 128, O], FP32, tag="ps_y")
            y_view = y.rearrange("p (b o) -> p b o", o=O)
            # block 0 (cols 17..145) only needs chunk 0's m; blocks 1-3 need
            # both chunks -> split 1 + 3 so the first DMA launches early
            for half, blocks in ((0, (0,)), (1, (1, 2, 3))):
                c0 = half * CH
                nc.vector.tensor_mul(mfin[:, 1 + c0:1 + c0 + CH],
                                     ufin[half], sfin[:, c0:c0 + CH])
                for b in blocks:
                    xs = slice(W + 1 + b * 128, W + 1 + (b + 1) * 128)
                    nc.tensor.matmul(ps_y[:, b, :], x_sb[:, xs], wox65,
                                     start=True, stop=False)
                    nc.tensor.matmul(ps_y[:, b, :], mfin[:, xs], woutT,
                                     start=False, stop=True)
                hb = slice(blocks[0], blocks[-1] + 1)
                nc.scalar.activation(ysb[:, hb, :], ps_y[:, hb, :], AF.Copy)
                nc.sync.dma_start(out=y_view[:, hb, :], in_=ysb[:, hb, :])

    nc.compile()
    return nc


_PROGRAM = None


def _get_program():
    global _PROGRAM
    if _PROGRAM is None:
        _PROGRAM = _build_program()
    return _PROGRAM


def _prepare_in_maps(inputs):
    x = np.ascontiguousarray(np.asarray(inputs["inputs"], dtype=np.float64)[63])
    W_in = np.asarray(inputs["W_in"], dtype=np.float64)
    b_in = np.asarray(inputs["b_in"], dtype=np.float64)
    W_g = np.asarray(inputs["W_g"], dtype=np.float64)
    b_g = np.asarray(inputs["b_g"], dtype=np.float64)
    W_out = np.asarray(inputs["W_out"], dtype=np.float64)
    b_out = np.asarray(inputs["b_out"], dtype=np.float64)

    Wg_h = W_g[:, :H]
    Wg_z = W_g[:, H:]

    c16 = np.zeros((128, NC16), np.float16)

    def put(name, v):
        c0, n_ = _C16_COLS[name]
        c16[:v.shape[0], c0:c0 + n_] = v.astype(np.float16)

    Wq = Wg_z + Wg_h
    put("q65", np.concatenate([(Wq @ W_in).T, (Wq @ b_in + b_g)[None, :]], 0))
    win65 = np.concatenate([W_in.T, b_in[None, :]], 0)
    put("winp", win65)
    put("winn", -win65)
    put("wghh", Wg_h.T)
    put("woutT", W_out.T)
    put("wox65", np.concatenate([(W_out @ W_in).T,
                                 (W_out @ b_in + b_out)[None, :]], 0))

    # x padded with W+1 leading and 1 trailing zero rows plus a ones-row
    # that carries the biases through the matmuls (zeroed where t < 0 or
    # t >= T)
    xpad = np.zeros((W + 1 + T + 1, I + 1), np.float16)
    xpad[W + 1:W + 1 + T, :I] = x.astype(np.float16)
    xpad[W + 1:W + 1 + T, I] = 1.0

    in_maps = []
    for k in range(NCORES):
        lo = k * TLOC
        xk = np.ascontiguousarray(xpad[lo:lo + NZ].T)
        in_maps.append({"xT": xk, "c16": c16})
    return in_maps


def _run(in_maps, **kwargs):
    nc = _get_program()
    return run_bass_kernel_spmd(nc, in_maps, list(range(NCORES)), **kwargs)


def _unshuffle(res):
    # per-core y arrives as [128, 4*O] (partition-contiguous); unshuffle to
    # [TLOC, O] with t = b*128 + p
    return np.concatenate(
        [res.results[k]["y"].reshape(128, TLOC // 128, O)
         .transpose(1, 0, 2).reshape(TLOC, O) for k in range(NCORES)],
        axis=0)


def kernel(**inputs):
    y = _unshuffle(_run(_prepare_in_maps(inputs)))
    return np.ascontiguousarray(y.astype(np.float32))


if __name__ == "__main__":
    d = np.load("/root/problem/inputs.npz")
    out = kernel(**{k: d[k] for k in d.files})
    exp = np.load("/root/problem/expected.npy")
    err = np.abs(out - exp).max()
    print("absmax err vs expected:", err, " rel:", err / np.abs(exp).max())
